# revision 33
# baseline (speedup 1.0000x reference)
"""AttentionSequencePoolingLayer (DIN attention) on 8 trn2 NeuronCores.

Data-parallel over batch: B=2048 -> 256 per core, processed as 64
"supergroups" of 4 batches (2 groups of 2).

Math per (b,t):  att = concat([q,k,q-k,q*k]) @ W1 + b1
  With W1 = [W1a;W1b;W1c;W1d] row blocks:
    att = k@(W1b-W1c) + (q*k)@W1d + (q@(W1a+W1c) + b1)
        = k@Wf_b + U_b          with Wf_b = (W1b-W1c) + diag(q_b)@W1d

Device layout: masked bf16 keys are loaded in two layouts -- kaug
([128, BL*T] feature-major, row 64 = ones, rows 65..127 = zeros so the
128-row weights trigger the compiler's fast-weight-load) feeding L1, and
kst ([128, *]: even batch on partitions 0..63, odd on 64..127) feeding
the pooling reduce.  L1: per-batch lhsT = [Wf_b; U_b] (the U bias rides
the matmul via the ones row, so sigmoid needs no per-batch bias and
activations merge into big instructions).  h1 = sigmoid(psum);
h2 = sigmoid(W2^T h1 + b2) stacked even/odd via tile_position; scores
via a block W3 (fp16) that broadcasts s_even to partitions 0..63 and
s_odd to 64..127; pooling is one fused DVE affine_mul_reduce per group
((s + b3) * mK summed over t; mask pre-folded into keys host-side so
padded positions contribute 0).  DMAs are batched 8 supergroups per
chunk (~1 MB) to stay in the efficient descriptor regime.
"""
import numpy as np
import ml_dtypes

import concourse.bacc as bacc
import concourse.bass as bass
import concourse.mybir as mybir
import concourse.tile as tile
from concourse.bass_utils import run_bass_kernel_spmd

B, T, E = 2048, 200, 64
H1, H2 = 80, 40
NCORES = 8
BL = B // NCORES          # 256 batches per core
NG = BL // 2              # 128 groups of 2 batches
NSG = BL // 4             # 64 supergroups of 4 batches
CH = 8                    # supergroups per DMA chunk

BF16 = ml_dtypes.bfloat16

_cache = {}

# opt-in profiling knobs (test.py sets these; harness leaves defaults)
TRACE = False
TRACE_KW = {}
LAST_RESULT = None


def _build(b3f: float):
    nc = bacc.Bacc(None, target_bir_lowering=False)
    f32 = mybir.dt.float32
    f32r = mybir.dt.float32r
    fp16 = mybir.dt.float16
    bf16 = mybir.dt.bfloat16
    SIG = mybir.ActivationFunctionType.Sigmoid

    kaug_d = nc.dram_tensor("kaug", [128, BL * T], bf16, kind="ExternalInput")
    kst_d = nc.dram_tensor("kst", [128, (BL // 2) * T], bf16, kind="ExternalInput")
    wfu_d = nc.dram_tensor("wfu", [128, BL * H1], bf16, kind="ExternalInput")
    w2e_d = nc.dram_tensor("w2e", [H1, 64], bf16, kind="ExternalInput")
    w2b_d = nc.dram_tensor("w2b", [H1, 64], bf16, kind="ExternalInput")
    w3blk_d = nc.dram_tensor("w3blk", [128, 128], fp16, kind="ExternalInput")
    b2c_d = nc.dram_tensor("b2c", [128, 1], f32, kind="ExternalInput")
    acc_d = nc.dram_tensor("acc", [128, NG], f32, kind="ExternalOutput")

    with tile.TileContext(nc) as tc:
        with (
            tc.tile_pool(name="const", bufs=1) as const,
            tc.tile_pool(name="keys", bufs=2) as keys_p,
            tc.tile_pool(name="wf", bufs=2) as wf_p,
            tc.tile_pool(name="act", bufs=3) as act_p,
            tc.tile_pool(name="p1", bufs=2, space=bass.MemorySpace.PSUM) as p1p,
            tc.tile_pool(name="p2", bufs=2, space=bass.MemorySpace.PSUM) as p2p,
            tc.tile_pool(name="p3", bufs=2, space=bass.MemorySpace.PSUM) as p3p,
        ):
            w2e_s = const.tile([H1, 64], bf16)
            w2b_s = const.tile([H1, 64], bf16)
            w3blk_s = const.tile([128, 128], fp16)
            b2c_s = const.tile([128, 1], f32)
            acc_s = const.tile([128, NG], f32)
            nc.sync.dma_start(w2e_s[:], w2e_d[:])
            nc.sync.dma_start(w2b_s[:], w2b_d[:])
            nc.sync.dma_start(w3blk_s[:], w3blk_d[:])
            nc.sync.dma_start(b2c_s[:], b2c_d[:])

            sched = [(c * CH, CH) for c in range(NSG // CH)]
            for s0, sz in sched:
                ktC = keys_p.tile([128, CH * 4 * T], bf16, tag="kt")
                k2C = keys_p.tile([128, CH * 2 * T], bf16, tag="k2")
                wfC = wf_p.tile([128, CH * 4 * H1], bf16, tag="wf")
                nc.sync.dma_start(
                    ktC[:, 0 : sz * 4 * T],
                    kaug_d[:, s0 * 4 * T : (s0 + sz) * 4 * T])
                nc.gpsimd.dma_start(
                    k2C[:, 0 : sz * 2 * T],
                    kst_d[:, s0 * 2 * T : (s0 + sz) * 2 * T])
                nc.sync.dma_start(
                    wfC[:, 0 : sz * 4 * H1],
                    wfu_d[:, s0 * 4 * H1 : (s0 + sz) * 4 * H1])
                for jl in range(sz):
                    j = s0 + jl
                    kt = ktC[:, jl * 4 * T : (jl + 1) * 4 * T]
                    k2 = k2C[:, jl * 2 * T : (jl + 1) * 2 * T]
                    wf = wfC[:, jl * 4 * H1 : (jl + 1) * 4 * H1]

                    # L1: att = [Wf_b; U_b]^T @ [k; 1] per batch; ones row
                    # baked into kaug carries U.  PSUM tile is 2 full banks;
                    # batches at cols 0,200,512,712 stay inside one bank.
                    # Batch order [0,2,1,3]: evens land in h1 cols 0..399,
                    # odds in 400..799 so L2 gets contiguous rhs slices.
                    p1 = p1p.tile([H1, 1024], f32, tag="p1")
                    for b, bb in enumerate((0, 2, 1, 3)):
                        c0 = b * T + (b // 2) * 112
                        nc.tensor.matmul(
                            p1[:, c0 : c0 + T],
                            wf[:, bb * H1 : (bb + 1) * H1],
                            kt[:, bb * T : (bb + 1) * T],
                            start=True,
                            stop=True,
                        )
                    h1 = act_p.tile([H1, 4 * T], bf16, tag="h1")
                    p1v = p1[:, :].rearrange("p (a b) -> p a b", a=2)[:, :, 0:400]
                    h1v = h1[:, :].rearrange("p (a b) -> p a b", a=2)
                    nc.scalar.activation(h1v, p1v, SIG)

                    # L2: h2 stacked per group: even batch -> rows 0..63 (w2e
                    # has zero cols 40..63), odd batch -> rows 64..127
                    p2 = p2p.tile([128, 2 * T], f32, tag="p2",
                                  padded_shape=[128, 512])
                    nc.tensor.matmul(
                        p2[0:64, :],
                        w2e_s[:],
                        h1[:, 0 : 2 * T],
                        start=True,
                        stop=True,
                    )
                    nc.tensor.matmul(
                        p2[64:128, :],
                        w2b_s[:],
                        h1[:, 2 * T : 4 * T],
                        start=True,
                        stop=True,
                        tile_position=(0, 64),
                    )
                    h2 = act_p.tile([128, 2 * T], fp16, tag="h2")
                    nc.scalar.activation(h2[:], p2[:], SIG, bias=b2c_s[:, 0:1])

                    # L3: scores broadcast: rows 0..63 = s_even, 64..127 = s_odd
                    p3 = p3p.tile([128, 2 * T], f32, tag="p3",
                                  padded_shape=[128, 512])
                    nc.tensor.matmul(
                        p3[:, :],
                        w3blk_s[:],
                        h2[:, :],
                        start=True,
                        stop=True,
                    )

                    # pooling: acc[:, g] = sum_t (s + b3) * mK  (fused DVE,
                    # custom ant-dve op; padded positions have mK == 0)
                    sc = act_p.tile([128, 2 * T], bf16, tag="sc")
                    for gp in range(2):
                        g = 2 * j + gp
                        nc.vector.affine_mul_reduce(
                            out=sc[:, gp * T : (gp + 1) * T],
                            accum_out=acc_s[:, g : g + 1],
                            in0=p3[:, gp * T : (gp + 1) * T],
                            in1=k2[:, gp * T : (gp + 1) * T],
                            scale=1.0,
                            bias=b3f,
                        )

            nc.sync.dma_start(acc_d[:], acc_s[:])

    nc.compile()
    return nc


def _prep_inputs(query, keys, keys_length, W1, b1, W2, b2, W3, b3):
    """Host-side folding; returns (in_maps, b3f)."""
    query = np.asarray(query, np.float32)
    keys = np.asarray(keys, np.float32)
    keys_length = np.asarray(keys_length, np.int32)
    W1 = np.asarray(W1, np.float32); b1 = np.asarray(b1, np.float32)
    W2 = np.asarray(W2, np.float32); b2 = np.asarray(b2, np.float32)
    W3 = np.asarray(W3, np.float32); b3 = np.asarray(b3, np.float32)

    A = W1[0:E] + W1[2 * E : 3 * E]          # q coeff
    Bw = W1[E : 2 * E] - W1[2 * E : 3 * E]   # k coeff
    C = W1[3 * E : 4 * E]                    # q*k coeff

    q2 = query[:, 0, :]                      # [B, E]
    U = (q2 @ A + b1).astype(BF16)           # [B, H1]
    Wf = (Bw[None, :, :] + q2[:, :, None] * C[None, :, :]).astype(BF16)
    mask = (np.arange(T)[None, :] < keys_length).astype(np.float32)
    mk_all = (keys * mask[:, :, None]).astype(BF16)      # [B, T, E]
    b3f = float(b3.reshape(-1)[0])

    w2e = np.zeros((H1, 64), np.float32); w2e[:, 0:H2] = W2
    w3blk = np.zeros((128, 128), np.float32)
    w3blk[0:H2, 0:64] = np.broadcast_to(W3, (H2, 64))
    w3blk[64 : 64 + H2, 64:128] = np.broadcast_to(W3, (H2, 64))
    b2c = np.zeros((128, 1), np.float32)
    b2c[0:H2, 0] = b2; b2c[64 : 64 + H2, 0] = b2
    in_maps = []
    for c in range(NCORES):
        s2 = slice(c * BL, (c + 1) * BL)
        mk = mk_all[s2]                                  # [BL, T, E] bf16
        kfm = np.ascontiguousarray(
            mk.transpose(2, 0, 1).reshape(E, BL * T))
        kaug = np.zeros((128, BL * T), BF16)             # rows 65..127 zero
        kaug[0:E] = kfm
        kaug[E] = 1
        kst = np.concatenate(
            [mk[0::2].transpose(2, 0, 1).reshape(E, (BL // 2) * T),
             mk[1::2].transpose(2, 0, 1).reshape(E, (BL // 2) * T)],
            axis=0)                                      # [128, BL/2*T]
        wfu_c = np.concatenate(
            [Wf[s2], U[s2][:, None, :]], axis=1)         # [BL, 65, H1]
        wfu = np.zeros((128, BL * H1), BF16)             # rows 65..127 zero
        wfu[0 : E + 1] = wfu_c.transpose(1, 0, 2).reshape(E + 1, BL * H1)
        in_maps.append({
            "kaug": np.ascontiguousarray(kaug),
            "kst": np.ascontiguousarray(kst),
            "wfu": wfu,
            "w2e": w2e.astype(BF16),
            "w2b": w2e.astype(BF16),
            "w3blk": w3blk.astype(np.float16),
            "b2c": b2c,
        })
    return in_maps, b3f


def kernel(query, keys, keys_length, W1, b1, W2, b2, W3, b3):
    in_maps, b3f = _prep_inputs(
        query, keys, keys_length, W1, b1, W2, b2, W3, b3)

    if _cache.get("b3f") != b3f:
        _cache["nc"] = _build(b3f)
        _cache["b3f"] = b3f
    nc = _cache["nc"]

    res = run_bass_kernel_spmd(
        nc, in_maps, list(range(NCORES)), trace=TRACE, **TRACE_KW
    )
    global LAST_RESULT
    LAST_RESULT = res
    outs = []
    for r in res.results:
        acc = np.asarray(r["acc"], np.float32)           # [128, NG]
        outs.append(acc.reshape(2, E, NG).transpose(2, 0, 1).reshape(BL, E))
    return np.concatenate(outs, 0).reshape(B, 1, E).astype(np.float32)
